# revision 16
# baseline (speedup 1.0000x reference)
"""DenseCapsule routing (2 iterations) on 8 Trainium2 cores.

Sharding: data-parallel over batch (8 batch elems per core, W fully
replicated and STREAMED from DRAM).  Routing is independent per batch
element, so there are NO collectives - each core's span is just its own
work, which sidesteps the ~60us cross-core launch skew that any
AllReduce would have to wait out.

Math (ITERATIONS=2, v0=0 => logits after iter1 are 0, cc1 = 1/K):
  u[b,k,c,i]   = sum_j W[k,c,i,j] x[b,c,j]
  v1           = squash(sum_c u / K)
  a[b,k,c]     = sum_i u[b,k,c,i] v1[b,k,i]        (logits for iter 2)
  cc           = softmax_k(a)
  v2           = squash(sum_c cc[b,k,c] u[b,k,c,i])   -> output

Per-core layouts (host-prepped, BL = 8 local batch elems):
  wt    [(c,j)=16384, (i,k)=512]     u-matmul rhs + s1 rhs (streamed)
  xtall [(c'16,j8)=128, t=128, BL]   s1 lhsT per c-subtile t
  xdo2  [g2=16][(c'16,j8), s=8, (c16,b8)=128]
        block-diag x: = x[b, 128*g2+16*s+c, j] * (c==c')
        u-matmul lhsT -> psum_u[(c,b), (i,k)] = u[b,k,c,i]
  obd8  [(c16,b'8)=128, BL]          ones block-diag: delta(b'==b)
        s2 reduction lhsT: psum_s2[b,(i,k)] += sum_c tmp2[(c,b'),(i,k)]

Phase 1 streams W: per super-tile (128 c's) the PE does 8 u-matmuls +
8 s1-matmuls while ACT/DVE evict u to SBUF (u_all, 128KB/partition).
Phase 2 (after squash(s1) -> v1) runs the routing chain per super-tile
with the elementwise work split DVE/Pool and the c-reduction on PE.
"""

import numpy as np

import concourse.bacc as bacc
import concourse.bass as bass
import concourse.tile as tile
from concourse import mybir
from concourse._compat import with_exitstack
from concourse.bass_utils import run_bass_kernel_spmd

NC = 8
B = 64
BL = B // NC        # 8 local batch elements
C = 2048
J = 8
K = 32
I = 16
G2 = 16             # super-tiles of 128 c's
NT = C // 16        # 128 c-subtiles (16 c's -> 128 (c,j) rows)
KI = K * I          # 512
EPS = 1e-7

F32 = mybir.dt.float32
BF16 = mybir.dt.bfloat16

TRACE = False           # test.py sets True to capture NTFF timing
LAST_RESULTS = None     # BassKernelResults of the last run


def _bcast_last(ap, n):
    """Append a stride-0 dim of size n to an AP (free-dim broadcast)."""
    return bass.AP(tensor=ap.tensor, offset=ap.offset, ap=[*ap.ap, [0, n]])


def _bcast_ins(ap, n):
    """Insert a stride-0 dim of size n BEFORE the last free dim, keeping
    the innermost run packed (enables the DVE 2x perf mode)."""
    return bass.AP(tensor=ap.tensor, offset=ap.offset,
                   ap=[*ap.ap[:-1], [0, n], ap.ap[-1]])


def _squash(nc, pool, eps_t, s_sb, pre, out_dt=F32):
    """v = squash(pre * s_sb) for s_sb [BL, (i,k)] f32, squash over i."""
    s3 = s_sb[:].rearrange("p (i k) -> p i k", k=K)
    sq = pool.tile([BL, I, K], F32, tag="sq_sq")
    nc.vector.tensor_mul(sq[:], s3, s3)
    t1 = pool.tile([BL, 8, K], F32, tag="sq_t1")
    nc.vector.tensor_add(t1[:], sq[:, 0:8, :], sq[:, 8:16, :])
    t2 = pool.tile([BL, 4, K], F32, tag="sq_t2")
    nc.vector.tensor_add(t2[:], t1[:, 0:4, :], t1[:, 4:8, :])
    t3 = pool.tile([BL, 2, K], F32, tag="sq_t3")
    nc.vector.tensor_add(t3[:], t2[:, 0:2, :], t2[:, 2:4, :])
    n0 = pool.tile([BL, K], F32, tag="sq_n0")
    nc.vector.tensor_add(n0[:], t3[:, 0, :], t3[:, 1, :])
    sn = pool.tile([BL, K], F32, tag="sq_sn")
    nc.scalar.mul(sn[:], n0[:], pre * pre)          # |s|^2
    rt = pool.tile([BL, K], F32, tag="sq_rt")
    nc.scalar.activation(rt[:], sn[:], mybir.ActivationFunctionType.Sqrt,
                         bias=eps_t[:], scale=1.0)  # sqrt(|s|^2 + eps)
    dn = pool.tile([BL, K], F32, tag="sq_dn")
    nc.scalar.add(dn[:], sn[:], 1.0)                # 1 + |s|^2
    dd = pool.tile([BL, K], F32, tag="sq_dd")
    nc.vector.tensor_mul(dd[:], dn[:], rt[:])
    rc = pool.tile([BL, K], F32, tag="sq_rc")
    nc.vector.reciprocal(rc[:], dd[:])
    f0 = pool.tile([BL, K], F32, tag="sq_f0")
    nc.vector.tensor_mul(f0[:], sn[:], rc[:])
    g0 = pool.tile([BL, K], F32, tag="sq_g0")
    nc.scalar.mul(g0[:], f0[:], pre)                # scale applied to raw s_sb
    v = pool.tile([BL, I, K], out_dt, tag="sq_v")
    nc.vector.tensor_mul(v[:], s3, _bcast_ins(g0[:], I))
    return v


@with_exitstack
def _body(ctx, tc, wt, xtall, xdo2, obd8, out_d):
    nc = tc.nc
    singles = ctx.enter_context(tc.tile_pool(name="singles", bufs=1))
    wtp = ctx.enter_context(tc.tile_pool(name="wtp", bufs=2))
    xdop = ctx.enter_context(tc.tile_pool(name="xdop", bufs=3))
    psS = ctx.enter_context(tc.tile_pool(name="psS", bufs=1, space="PSUM"))
    psU = ctx.enter_context(tc.tile_pool(name="psU", bufs=3, space="PSUM"))
    work = ctx.enter_context(tc.tile_pool(name="work", bufs=1))
    workB = ctx.enter_context(tc.tile_pool(name="workB", bufs=2))
    sm = ctx.enter_context(tc.tile_pool(name="sm", bufs=2))
    sm1 = ctx.enter_context(tc.tile_pool(name="sm1", bufs=1))
    dram = ctx.enter_context(tc.tile_pool(name="dram", bufs=1, space="DRAM"))

    xtall_sb = singles.tile([128, NT, BL], BF16)
    nc.sync.dma_start(out=xtall_sb[:], in_=xtall)
    obd_sb = singles.tile([128, BL], BF16)
    nc.sync.dma_start(out=obd_sb[:], in_=obd8)
    eps_t = singles.tile([BL, 1], F32)
    nc.vector.memset(eps_t[:], EPS)
    u_all = singles.tile([128, G2, 8, KI], BF16)
    v1rep = singles.tile([128, KI], BF16)
    v1d = dram.tile([BL, KI], BF16, name="v1d")

    # ---- phase 1: stream W; u-matmuls + s1-matmuls; evict u ----
    ps_s1 = psS.tile([BL, KI], F32)
    wt_ap = wt  # [C*J, KI] dram AP
    for g2 in range(G2):
        wt_t = wtp.tile([128, 8, KI], BF16, tag="wt")
        src = bass.AP(tensor=wt_ap.tensor,
                      offset=wt_ap.offset + (1024 * g2) * KI,
                      ap=[[KI, 128], [128 * KI, 8], [1, KI]])
        nc.sync.dma_start(out=wt_t[:], in_=src)
        xdo_t = xdop.tile([128, 8, 128], BF16, tag="xdo")
        nc.gpsimd.dma_start(out=xdo_t[:], in_=xdo2[g2])
        for sp in range(4):                    # pairs of c-subtiles
            ps_u = psU.tile([128, 2, KI], F32, tag="psu")
            for h in range(2):
                s = 2 * sp + h
                t = 8 * g2 + s
                nc.tensor.matmul(ps_u[:, h, :], lhsT=xdo_t[:, s, :],
                                 rhs=wt_t[:, s, :], start=True, stop=True)
                nc.tensor.matmul(ps_s1[:], lhsT=xtall_sb[:, t, :],
                                 rhs=wt_t[:, s, :],
                                 start=(t == 0), stop=(t == NT - 1))
            dst = u_all[:, g2, 2 * sp:2 * sp + 2, :]
            if (4 * g2 + sp) % 2 == 0:
                nc.scalar.copy(dst, ps_u[:])
            else:
                nc.vector.tensor_copy(dst, ps_u[:])

    # ---- v1 = squash(s1 / K); replicate to all 128 partitions ----
    s1sb = sm1.tile([BL, KI], F32, tag="s1sb")
    nc.scalar.copy(s1sb[:], ps_s1[:])
    v1 = _squash(nc, sm1, eps_t, s1sb, 1.0 / K)
    v1b = sm1.tile([BL, KI], BF16, tag="v1b")
    nc.vector.tensor_copy(v1b[:], v1[:].rearrange("p i k -> p (i k)"))
    nc.sync.dma_start(out=v1d[:], in_=v1b[:])
    v1d_ap = v1d[:]
    rep_src = bass.AP(tensor=v1d_ap.tensor, offset=v1d_ap.offset,
                      ap=[[0, 16], [KI, 8], [1, KI]])
    nc.sync.dma_start(out=v1rep[:], in_=rep_src)

    # ---- phase 2: routing chain per super-tile ----
    ps_s2 = psS.tile([BL, KI], F32)
    for g2 in range(G2):
        u_g = u_all[:, g2]                     # [128, 8, KI]
        tmp = work.tile([128, 8, KI], BF16, tag="tmp")
        nc.vector.tensor_mul(tmp[:], u_g, _bcast_ins(v1rep[:], 8))
        t4 = tmp[:].rearrange("p s (i k) -> p s i k", k=K)
        f1 = workB.tile([128, 8, 8, K], BF16, tag="f1")
        nc.vector.tensor_add(f1[:], t4[:, :, 0:8, :], t4[:, :, 8:16, :])
        f2 = sm.tile([128, 8, 4, K], BF16, tag="f2")
        nc.gpsimd.tensor_add(f2[:], f1[:, :, 0:4, :], f1[:, :, 4:8, :])
        f3 = sm.tile([128, 8, 2, K], BF16, tag="f3")
        nc.gpsimd.tensor_add(f3[:], f2[:, :, 0:2, :], f2[:, :, 2:4, :])
        a_t = sm.tile([128, 8, K], F32, tag="a")
        nc.gpsimd.tensor_add(a_t[:], f3[:, :, 0, :], f3[:, :, 1, :])
        e_t = sm.tile([128, 8, K], BF16, tag="e")
        nc.scalar.activation(e_t[:], a_t[:],
                             mybir.ActivationFunctionType.Exp, scale=1.0)
        den = sm.tile([128, 8], F32, tag="den")
        nc.vector.reduce_sum(den[:], e_t[:], axis=mybir.AxisListType.X)
        rcp = sm.tile([128, 8], F32, tag="rcp")
        nc.vector.reciprocal(rcp[:], den[:])
        cc = sm.tile([128, 8, K], BF16, tag="cc")
        nc.gpsimd.tensor_mul(cc[:], e_t[:], _bcast_last(rcp[:], K))
        tmp2 = work.tile([128, 8, I, K], BF16, tag="tmp2")
        nc.vector.tensor_mul(
            tmp2[:], u_g.rearrange("p s (i k) -> p s i k", k=K),
            _bcast_ins(cc[:], I))
        for s in range(8):
            nc.tensor.matmul(
                ps_s2[:], lhsT=obd_sb[:],
                rhs=tmp2[:, s, :, :].rearrange("p i k -> p (i k)"),
                start=(g2 == 0 and s == 0), stop=(g2 == G2 - 1 and s == 7))

    # ---- v2 = squash(s2) -> out ----
    s2sb = sm1.tile([BL, KI], F32, tag="s2sb")
    nc.scalar.copy(s2sb[:], ps_s2[:])
    v2 = _squash(nc, sm1, eps_t, s2sb, 1.0)
    nc.sync.dma_start(out=out_d, in_=v2[:].rearrange("p i k -> p (i k)"))


_PROG = None


def _get_program():
    global _PROG
    if _PROG is None:
        nc = bacc.Bacc("TRN2", target_bir_lowering=False, debug=False,
                       num_devices=NC)
        wt_d = nc.dram_tensor("wt", [C * J, KI], BF16, kind="ExternalInput")
        xtall_d = nc.dram_tensor("xtall", [128, NT, BL], BF16,
                                 kind="ExternalInput")
        xdo2_d = nc.dram_tensor("xdo2", [G2, 128, 8, 128], BF16,
                                kind="ExternalInput")
        obd8_d = nc.dram_tensor("obd8", [128, BL], BF16, kind="ExternalInput")
        out_d = nc.dram_tensor("out", [BL, KI], F32, kind="ExternalOutput")
        with tile.TileContext(nc) as tc:
            _body(tc, wt_d[:], xtall_d[:], xdo2_d[:], obd8_d[:], out_d[:])
        nc.compile()
        _PROG = nc
    return _PROG


def _prep_inputs(x, W):
    import ml_dtypes
    bf = ml_dtypes.bfloat16
    # W [K, C, I, J] -> wt [(c,j), (i,k)]   (shared by all cores)
    wt = np.ascontiguousarray(W.transpose(1, 3, 2, 0)).reshape(C * J, KI)
    wt = wt.astype(bf)
    obd8 = np.zeros((16, BL, BL), np.float32)
    for b in range(BL):
        obd8[:, b, b] = 1.0
    obd8 = obd8.reshape(128, BL).astype(bf)
    in_maps = []
    for m in range(NC):
        xs = x[m * BL:(m + 1) * BL]                    # [BL, C, J]
        # xtall [(c'16,j8), t, b]
        xt5 = xs.reshape(BL, NT, 16, J)                # [b, t, c', j]
        xtall = np.ascontiguousarray(
            xt5.transpose(2, 3, 1, 0)).reshape(128, NT, BL).astype(bf)
        # xdo2 [g2, (c'16,j8), s, (c16,b8)]
        x6 = xs.reshape(BL, G2, 8, 16, J)              # [b, g2, s, c', j]
        xdo2 = np.zeros((G2, 16, J, 8, 16, BL), np.float32)
        for cp in range(16):
            # [b, g2, s, j] -> [g2, j, s, b]
            xdo2[:, cp, :, :, cp, :] = x6[:, :, :, cp, :].transpose(1, 3, 2, 0)
        xdo2 = xdo2.reshape(G2, 128, 8, 128).astype(bf)
        in_maps.append({"wt": wt, "xtall": xtall, "xdo2": xdo2, "obd8": obd8})
    return in_maps


def kernel(x, W):
    global LAST_RESULTS
    x = np.ascontiguousarray(np.asarray(x, np.float32))
    W = np.ascontiguousarray(np.asarray(W, np.float32))
    assert x.shape == (B, C, J) and W.shape == (K, C, I, J)
    nc = _get_program()
    in_maps = _prep_inputs(x, W)
    res = run_bass_kernel_spmd(nc, in_maps, core_ids=list(range(NC)),
                               trace=TRACE)
    LAST_RESULTS = res
    out = np.empty((B, K, I), np.float32)
    for m in range(NC):
        vm = np.asarray(res.results[m]["out"], np.float32).reshape(BL, I, K)
        out[m * BL:(m + 1) * BL] = vm.transpose(0, 2, 1)
    return np.ascontiguousarray(out)


# revision 17
# speedup vs baseline: 1.0373x; 1.0373x over previous
"""DenseCapsule routing (2 iterations) on 8 Trainium2 cores.

Sharding: data-parallel over batch (8 batch elems per core, W fully
replicated and STREAMED from DRAM).  Routing is independent per batch
element, so there are NO collectives - each core's span is just its own
work, which sidesteps the ~60us cross-core launch skew that any
AllReduce would have to wait out.

Math (ITERATIONS=2, v0=0 => logits after iter1 are 0, cc1 = 1/K):
  u[b,k,c,i]   = sum_j W[k,c,i,j] x[b,c,j]
  v1           = squash(sum_c u / K)
  a[b,k,c]     = sum_i u[b,k,c,i] v1[b,k,i]        (logits for iter 2)
  cc           = softmax_k(a)
  v2           = squash(sum_c cc[b,k,c] u[b,k,c,i])   -> output

Per-core layouts (host-prepped, BL = 8 local batch elems):
  wt    [(c,j)=16384, (i,k)=512]     u-matmul rhs + s1 rhs (streamed)
  xtall [(c'16,j8)=128, t=128, BL]   s1 lhsT per c-subtile t
  xdo2  [g2=16][(c'16,j8), s=8, (c16,b8)=128]
        block-diag x: = x[b, 128*g2+16*s+c, j] * (c==c')
        u-matmul lhsT -> psum_u[(c,b), (i,k)] = u[b,k,c,i]
  obd8  [(c16,b'8)=128, BL]          ones block-diag: delta(b'==b)
        s2 reduction lhsT: psum_s2[b,(i,k)] += sum_c tmp2[(c,b'),(i,k)]

Phase 1 streams W: per super-tile (128 c's) the PE does 8 u-matmuls +
8 s1-matmuls while ACT/DVE evict u to SBUF (u_all, 128KB/partition).
Phase 2 (after squash(s1) -> v1) runs the routing chain per super-tile
with the elementwise work split DVE/Pool and the c-reduction on PE.
"""

import numpy as np

import concourse.bacc as bacc
import concourse.bass as bass
import concourse.tile as tile
from concourse import mybir
from concourse._compat import with_exitstack
from concourse.bass_utils import run_bass_kernel_spmd

NC = 8
B = 64
BL = B // NC        # 8 local batch elements
C = 2048
J = 8
K = 32
I = 16
G2 = 16             # super-tiles of 128 c's
NT = C // 16        # 128 c-subtiles (16 c's -> 128 (c,j) rows)
KI = K * I          # 512
EPS = 1e-7

F32 = mybir.dt.float32
BF16 = mybir.dt.bfloat16

TRACE = False           # test.py sets True to capture NTFF timing
LAST_RESULTS = None     # BassKernelResults of the last run


def _bcast_last(ap, n):
    """Append a stride-0 dim of size n to an AP (free-dim broadcast)."""
    return bass.AP(tensor=ap.tensor, offset=ap.offset, ap=[*ap.ap, [0, n]])


def _bcast_ins(ap, n):
    """Insert a stride-0 dim of size n BEFORE the last free dim, keeping
    the innermost run packed (enables the DVE 2x perf mode)."""
    return bass.AP(tensor=ap.tensor, offset=ap.offset,
                   ap=[*ap.ap[:-1], [0, n], ap.ap[-1]])


def _squash(nc, pool, eps_t, s_sb, pre, out_dt=F32):
    """v = squash(pre * s_sb) for s_sb [BL, (i,k)] f32, squash over i."""
    s3 = s_sb[:].rearrange("p (i k) -> p i k", k=K)
    sq = pool.tile([BL, I, K], F32, tag="sq_sq")
    nc.vector.tensor_mul(sq[:], s3, s3)
    t1 = pool.tile([BL, 8, K], F32, tag="sq_t1")
    nc.vector.tensor_add(t1[:], sq[:, 0:8, :], sq[:, 8:16, :])
    t2 = pool.tile([BL, 4, K], F32, tag="sq_t2")
    nc.vector.tensor_add(t2[:], t1[:, 0:4, :], t1[:, 4:8, :])
    t3 = pool.tile([BL, 2, K], F32, tag="sq_t3")
    nc.vector.tensor_add(t3[:], t2[:, 0:2, :], t2[:, 2:4, :])
    n0 = pool.tile([BL, K], F32, tag="sq_n0")
    nc.vector.tensor_add(n0[:], t3[:, 0, :], t3[:, 1, :])
    sn = pool.tile([BL, K], F32, tag="sq_sn")
    nc.scalar.mul(sn[:], n0[:], pre * pre)          # |s|^2
    rt = pool.tile([BL, K], F32, tag="sq_rt")
    nc.scalar.activation(rt[:], sn[:], mybir.ActivationFunctionType.Sqrt,
                         bias=eps_t[:], scale=1.0)  # sqrt(|s|^2 + eps)
    dn = pool.tile([BL, K], F32, tag="sq_dn")
    nc.scalar.add(dn[:], sn[:], 1.0)                # 1 + |s|^2
    dd = pool.tile([BL, K], F32, tag="sq_dd")
    nc.vector.tensor_mul(dd[:], dn[:], rt[:])
    rc = pool.tile([BL, K], F32, tag="sq_rc")
    nc.vector.reciprocal(rc[:], dd[:])
    f0 = pool.tile([BL, K], F32, tag="sq_f0")
    nc.vector.tensor_mul(f0[:], sn[:], rc[:])
    g0 = pool.tile([BL, K], F32, tag="sq_g0")
    nc.scalar.mul(g0[:], f0[:], pre)                # scale applied to raw s_sb
    v = pool.tile([BL, I, K], out_dt, tag="sq_v")
    nc.vector.tensor_mul(v[:], s3, _bcast_ins(g0[:], I))
    return v


@with_exitstack
def _body(ctx, tc, wt, xtall, xdo2, obd8, out_d):
    nc = tc.nc
    singles = ctx.enter_context(tc.tile_pool(name="singles", bufs=1))
    wtp = ctx.enter_context(tc.tile_pool(name="wtp", bufs=2))
    xdop = ctx.enter_context(tc.tile_pool(name="xdop", bufs=3))
    psS = ctx.enter_context(tc.tile_pool(name="psS", bufs=1, space="PSUM"))
    psU = ctx.enter_context(tc.tile_pool(name="psU", bufs=3, space="PSUM"))
    work = ctx.enter_context(tc.tile_pool(name="work", bufs=1))
    workB = ctx.enter_context(tc.tile_pool(name="workB", bufs=2))
    sm = ctx.enter_context(tc.tile_pool(name="sm", bufs=2))
    sm1 = ctx.enter_context(tc.tile_pool(name="sm1", bufs=1))
    dram = ctx.enter_context(tc.tile_pool(name="dram", bufs=1, space="DRAM"))

    xtall_sb = singles.tile([128, NT, BL], BF16)
    nc.sync.dma_start(out=xtall_sb[:], in_=xtall)
    obd_sb = singles.tile([128, BL], BF16)
    nc.sync.dma_start(out=obd_sb[:], in_=obd8)
    eps_t = singles.tile([BL, 1], F32)
    nc.vector.memset(eps_t[:], EPS)
    u_all = singles.tile([128, G2, 8, KI], BF16)
    v1rep = singles.tile([128, KI], BF16)
    v1d = dram.tile([BL, KI], BF16, name="v1d")

    # ---- phase 1: stream W; u-matmuls + s1-matmuls; evict u ----
    ps_s1 = psS.tile([BL, KI], F32)
    wt_ap = wt  # [C*J, KI] dram AP
    for g2 in range(G2):
        wt_t = wtp.tile([128, 8, KI], BF16, tag="wt")
        src = bass.AP(tensor=wt_ap.tensor,
                      offset=wt_ap.offset + (1024 * g2) * KI,
                      ap=[[KI, 128], [128 * KI, 8], [1, KI]])
        nc.sync.dma_start(out=wt_t[:], in_=src)
        xdo_t = xdop.tile([128, 8, 128], BF16, tag="xdo")
        nc.gpsimd.dma_start(out=xdo_t[:], in_=xdo2[g2])
        for sp in range(4):                    # pairs of c-subtiles
            ps_u = psU.tile([128, 2, KI], F32, tag="psu")
            for h in range(2):
                s = 2 * sp + h
                t = 8 * g2 + s
                nc.tensor.matmul(ps_u[:, h, :], lhsT=xdo_t[:, s, :],
                                 rhs=wt_t[:, s, :], start=True, stop=True)
                nc.tensor.matmul(ps_s1[:], lhsT=xtall_sb[:, t, :],
                                 rhs=wt_t[:, s, :],
                                 start=(t == 0), stop=(t == NT - 1))
            dst = u_all[:, g2, 2 * sp:2 * sp + 2, :]
            if (4 * g2 + sp) % 2 == 0:
                nc.scalar.copy(dst, ps_u[:])
            else:
                nc.vector.tensor_copy(dst, ps_u[:])

    # ---- v1 = squash(s1 / K); replicate to all 128 partitions ----
    s1sb = sm1.tile([BL, KI], F32, tag="s1sb")
    nc.scalar.copy(s1sb[:], ps_s1[:])
    v1 = _squash(nc, sm1, eps_t, s1sb, 1.0 / K)
    v1b = sm1.tile([BL, KI], BF16, tag="v1b")
    nc.vector.tensor_copy(v1b[:], v1[:].rearrange("p i k -> p (i k)"))
    nc.sync.dma_start(out=v1d[:], in_=v1b[:])
    v1d_ap = v1d[:]
    rep_src = bass.AP(tensor=v1d_ap.tensor, offset=v1d_ap.offset,
                      ap=[[0, 16], [KI, 8], [1, KI]])
    nc.sync.dma_start(out=v1rep[:], in_=rep_src)

    # ---- phase 2: routing chain per super-tile, software-pipelined so
    # the DVE works on tile g+1's mul/tree while Pool/ACT chew tile g ----
    ps_s2 = psS.tile([BL, KI], F32)
    e_tiles = {}

    def stage_a(g2):
        u_g = u_all[:, g2]                     # [128, 8, KI]
        tmp = work.tile([128, 8, KI], BF16, tag="tmp")
        nc.vector.tensor_mul(tmp[:], u_g, _bcast_ins(v1rep[:], 8))
        t4 = tmp[:].rearrange("p s (i k) -> p s i k", k=K)
        f1 = workB.tile([128, 8, 8, K], BF16, tag="f1")
        nc.vector.tensor_add(f1[:], t4[:, :, 0:8, :], t4[:, :, 8:16, :])
        f2 = sm.tile([128, 8, 4, K], BF16, tag="f2")
        nc.gpsimd.tensor_add(f2[:], f1[:, :, 0:4, :], f1[:, :, 4:8, :])
        f3 = sm.tile([128, 8, 2, K], BF16, tag="f3")
        nc.gpsimd.tensor_add(f3[:], f2[:, :, 0:2, :], f2[:, :, 2:4, :])
        a_t = sm.tile([128, 8, K], F32, tag="a")
        nc.gpsimd.tensor_add(a_t[:], f3[:, :, 0, :], f3[:, :, 1, :])
        e_t = sm.tile([128, 8, K], BF16, tag="e")
        nc.scalar.activation(e_t[:], a_t[:],
                             mybir.ActivationFunctionType.Exp, scale=1.0)
        e_tiles[g2] = e_t

    def stage_b(g2):
        u_g = u_all[:, g2]
        e_t = e_tiles.pop(g2)
        den = sm.tile([128, 8], F32, tag="den")
        nc.vector.reduce_sum(den[:], e_t[:], axis=mybir.AxisListType.X)
        rcp = sm.tile([128, 8], F32, tag="rcp")
        nc.vector.reciprocal(rcp[:], den[:])
        cc = sm.tile([128, 8, K], BF16, tag="cc")
        nc.gpsimd.tensor_mul(cc[:], e_t[:], _bcast_last(rcp[:], K))
        tmp2 = workB.tile([128, 8, I, K], BF16, tag="tmp2")
        nc.vector.tensor_mul(
            tmp2[:], u_g.rearrange("p s (i k) -> p s i k", k=K),
            _bcast_ins(cc[:], I))
        for s in range(8):
            nc.tensor.matmul(
                ps_s2[:], lhsT=obd_sb[:],
                rhs=tmp2[:, s, :, :].rearrange("p i k -> p (i k)"),
                start=(g2 == 0 and s == 0), stop=(g2 == G2 - 1 and s == 7))

    for g2 in range(G2):
        stage_a(g2)
        if g2 > 0:
            stage_b(g2 - 1)
    stage_b(G2 - 1)

    # ---- v2 = squash(s2) -> out ----
    s2sb = sm1.tile([BL, KI], F32, tag="s2sb")
    nc.scalar.copy(s2sb[:], ps_s2[:])
    v2 = _squash(nc, sm1, eps_t, s2sb, 1.0)
    nc.sync.dma_start(out=out_d, in_=v2[:].rearrange("p i k -> p (i k)"))


_PROG = None


def _get_program():
    global _PROG
    if _PROG is None:
        nc = bacc.Bacc("TRN2", target_bir_lowering=False, debug=False,
                       num_devices=NC)
        wt_d = nc.dram_tensor("wt", [C * J, KI], BF16, kind="ExternalInput")
        xtall_d = nc.dram_tensor("xtall", [128, NT, BL], BF16,
                                 kind="ExternalInput")
        xdo2_d = nc.dram_tensor("xdo2", [G2, 128, 8, 128], BF16,
                                kind="ExternalInput")
        obd8_d = nc.dram_tensor("obd8", [128, BL], BF16, kind="ExternalInput")
        out_d = nc.dram_tensor("out", [BL, KI], F32, kind="ExternalOutput")
        with tile.TileContext(nc) as tc:
            _body(tc, wt_d[:], xtall_d[:], xdo2_d[:], obd8_d[:], out_d[:])
        nc.compile()
        _PROG = nc
    return _PROG


def _prep_inputs(x, W):
    import ml_dtypes
    bf = ml_dtypes.bfloat16
    # W [K, C, I, J] -> wt [(c,j), (i,k)]   (shared by all cores)
    wt = np.ascontiguousarray(W.transpose(1, 3, 2, 0)).reshape(C * J, KI)
    wt = wt.astype(bf)
    obd8 = np.zeros((16, BL, BL), np.float32)
    for b in range(BL):
        obd8[:, b, b] = 1.0
    obd8 = obd8.reshape(128, BL).astype(bf)
    in_maps = []
    for m in range(NC):
        xs = x[m * BL:(m + 1) * BL]                    # [BL, C, J]
        # xtall [(c'16,j8), t, b]
        xt5 = xs.reshape(BL, NT, 16, J)                # [b, t, c', j]
        xtall = np.ascontiguousarray(
            xt5.transpose(2, 3, 1, 0)).reshape(128, NT, BL).astype(bf)
        # xdo2 [g2, (c'16,j8), s, (c16,b8)]
        x6 = xs.reshape(BL, G2, 8, 16, J)              # [b, g2, s, c', j]
        xdo2 = np.zeros((G2, 16, J, 8, 16, BL), np.float32)
        for cp in range(16):
            # [b, g2, s, j] -> [g2, j, s, b]
            xdo2[:, cp, :, :, cp, :] = x6[:, :, :, cp, :].transpose(1, 3, 2, 0)
        xdo2 = xdo2.reshape(G2, 128, 8, 128).astype(bf)
        in_maps.append({"wt": wt, "xtall": xtall, "xdo2": xdo2, "obd8": obd8})
    return in_maps


def kernel(x, W):
    global LAST_RESULTS
    x = np.ascontiguousarray(np.asarray(x, np.float32))
    W = np.ascontiguousarray(np.asarray(W, np.float32))
    assert x.shape == (B, C, J) and W.shape == (K, C, I, J)
    nc = _get_program()
    in_maps = _prep_inputs(x, W)
    res = run_bass_kernel_spmd(nc, in_maps, core_ids=list(range(NC)),
                               trace=TRACE)
    LAST_RESULTS = res
    out = np.empty((B, K, I), np.float32)
    for m in range(NC):
        vm = np.asarray(res.results[m]["out"], np.float32).reshape(BL, I, K)
        out[m * BL:(m + 1) * BL] = vm.transpose(0, 2, 1)
    return np.ascontiguousarray(out)


# revision 28
# speedup vs baseline: 1.0516x; 1.0137x over previous
"""DenseCapsule routing (2 iterations) on 8 Trainium2 cores.

Sharding: data-parallel over batch (8 batch elems per core, W fully
replicated and STREAMED from DRAM).  Routing is independent per batch
element, so there are NO collectives - each core's span is just its own
work, which sidesteps the ~60us cross-core launch skew that any
AllReduce would have to wait out.

Math (ITERATIONS=2, v0=0 => logits after iter1 are 0, cc1 = 1/K):
  u[b,k,c,i]   = sum_j W[k,c,i,j] x[b,c,j]
  v1           = squash(sum_c u / K)
  a[b,k,c]     = sum_i u[b,k,c,i] v1[b,k,i]        (logits for iter 2)
  cc           = softmax_k(a)
  v2           = squash(sum_c cc[b,k,c] u[b,k,c,i])   -> output

Per-core layouts (host-prepped, BL = 8 local batch elems):
  wt    [(c,j)=16384, (i,k)=512]     u-matmul rhs + s1 rhs (streamed)
  xtall [(c'16,j8)=128, t=128, BL]   s1 lhsT per c-subtile t
  xdo2  [g2=16][(c'16,j8), s=8, (c16,b8)=128]
        block-diag x: = x[b, 128*g2+16*s+c, j] * (c==c')
        u-matmul lhsT -> psum_u[(c,b), (i,k)] = u[b,k,c,i]
  obd8  [(c16,b'8)=128, BL]          ones block-diag: delta(b'==b)
        s2 reduction lhsT: psum_s2[b,(i,k)] += sum_c tmp2[(c,b'),(i,k)]

Phase 1 streams W: per super-tile (128 c's) the PE does 8 u-matmuls +
8 s1-matmuls while ACT/DVE evict u to SBUF (u_all, 128KB/partition).
Phase 2 (after squash(s1) -> v1) runs the routing chain per super-tile
with the elementwise work split DVE/Pool and the c-reduction on PE.
"""

import numpy as np

import concourse.bacc as bacc
import concourse.bass as bass
import concourse.tile as tile
from concourse import mybir
from concourse._compat import with_exitstack
from concourse.bass_utils import run_bass_kernel_spmd

NC = 8
B = 64
BL = B // NC        # 8 local batch elements
C = 2048
J = 8
K = 32
I = 16
G2 = 16             # super-tiles of 128 c's
NT = C // 16        # 128 c-subtiles (16 c's -> 128 (c,j) rows)
KI = K * I          # 512
EPS = 1e-7

F32 = mybir.dt.float32
BF16 = mybir.dt.bfloat16

TRACE = False           # test.py sets True to capture NTFF timing
LAST_RESULTS = None     # BassKernelResults of the last run


def _bcast_last(ap, n):
    """Append a stride-0 dim of size n to an AP (free-dim broadcast)."""
    return bass.AP(tensor=ap.tensor, offset=ap.offset, ap=[*ap.ap, [0, n]])


def _bcast_ins(ap, n):
    """Insert a stride-0 dim of size n BEFORE the last free dim, keeping
    the innermost run packed (enables the DVE 2x perf mode)."""
    return bass.AP(tensor=ap.tensor, offset=ap.offset,
                   ap=[*ap.ap[:-1], [0, n], ap.ap[-1]])


def _squash(nc, pool, eps_t, s_sb, pre, out_dt=F32):
    """v = squash(pre * s_sb) for s_sb [BL, (i,k)] f32, squash over i."""
    s3 = s_sb[:].rearrange("p (i k) -> p i k", k=K)
    sq = pool.tile([BL, I, K], F32, tag="sq_sq")
    nc.vector.tensor_mul(sq[:], s3, s3)
    t1 = pool.tile([BL, 8, K], F32, tag="sq_t1")
    nc.vector.tensor_add(t1[:], sq[:, 0:8, :], sq[:, 8:16, :])
    t2 = pool.tile([BL, 4, K], F32, tag="sq_t2")
    nc.vector.tensor_add(t2[:], t1[:, 0:4, :], t1[:, 4:8, :])
    t3 = pool.tile([BL, 2, K], F32, tag="sq_t3")
    nc.vector.tensor_add(t3[:], t2[:, 0:2, :], t2[:, 2:4, :])
    n0 = pool.tile([BL, K], F32, tag="sq_n0")
    nc.vector.tensor_add(n0[:], t3[:, 0, :], t3[:, 1, :])
    sn = pool.tile([BL, K], F32, tag="sq_sn")
    nc.scalar.mul(sn[:], n0[:], pre * pre)          # |s|^2
    rt = pool.tile([BL, K], F32, tag="sq_rt")
    nc.scalar.activation(rt[:], sn[:], mybir.ActivationFunctionType.Sqrt,
                         bias=eps_t[:], scale=1.0)  # sqrt(|s|^2 + eps)
    dn = pool.tile([BL, K], F32, tag="sq_dn")
    nc.scalar.add(dn[:], sn[:], 1.0)                # 1 + |s|^2
    dd = pool.tile([BL, K], F32, tag="sq_dd")
    nc.vector.tensor_mul(dd[:], dn[:], rt[:])
    rc = pool.tile([BL, K], F32, tag="sq_rc")
    nc.vector.reciprocal(rc[:], dd[:])
    f0 = pool.tile([BL, K], F32, tag="sq_f0")
    nc.vector.tensor_mul(f0[:], sn[:], rc[:])
    g0 = pool.tile([BL, K], F32, tag="sq_g0")
    nc.scalar.mul(g0[:], f0[:], pre)                # scale applied to raw s_sb
    v = pool.tile([BL, I, K], out_dt, tag="sq_v")
    nc.vector.tensor_mul(v[:], s3, _bcast_ins(g0[:], I))
    return v


@with_exitstack
def _body(ctx, tc, wt, xtall, xdo2, obd8, out_d):
    nc = tc.nc
    singles = ctx.enter_context(tc.tile_pool(name="singles", bufs=1))
    wtp = ctx.enter_context(tc.tile_pool(name="wtp", bufs=2))
    xdop = ctx.enter_context(tc.tile_pool(name="xdop", bufs=2))
    psS = ctx.enter_context(tc.tile_pool(name="psS", bufs=1, space="PSUM"))
    psU = ctx.enter_context(tc.tile_pool(name="psU", bufs=2, space="PSUM"))
    work = ctx.enter_context(tc.tile_pool(name="work", bufs=1))
    workB = ctx.enter_context(tc.tile_pool(name="workB", bufs=2))
    sm = ctx.enter_context(tc.tile_pool(name="sm", bufs=2))
    sm1 = ctx.enter_context(tc.tile_pool(name="sm1", bufs=1))
    dram = ctx.enter_context(tc.tile_pool(name="dram", bufs=1, space="DRAM"))

    xtall_sb = singles.tile([128, NT, BL], BF16)
    nc.sync.dma_start(out=xtall_sb[:], in_=xtall)
    obd_sb = singles.tile([128, BL], BF16)
    nc.sync.dma_start(out=obd_sb[:], in_=obd8)
    eps_t = singles.tile([BL, 1], F32)
    nc.vector.memset(eps_t[:], EPS)
    u_all = singles.tile([128, G2, 8, KI], BF16)
    v1rep = singles.tile([128, KI], BF16)
    v1d = dram.tile([BL, KI], BF16, name="v1d")

    # ---- phase 1: stream W; u-matmuls + s1-matmuls; evict u ----
    # s1 accumulates into two PSUM banks round-robin: back-to-back
    # accumulation into ONE bank serializes matmuls on the bank-RAW
    # hazard (~390ns each instead of ~215).
    ps_s1 = [psS.tile([BL, KI], F32, name=f"ps_s1_{h}") for h in range(2)]
    wt_ap = wt  # [C*J, KI] dram AP
    for g2 in range(G2):
        wt_t = wtp.tile([128, 8, KI], BF16, tag="wt")
        src = bass.AP(tensor=wt_ap.tensor,
                      offset=wt_ap.offset + (1024 * g2) * KI,
                      ap=[[KI, 128], [128 * KI, 8], [1, KI]])
        nc.sync.dma_start(out=wt_t[:], in_=src)
        xdo_t = xdop.tile([128, 8, 128], BF16, tag="xdo")
        nc.gpsimd.dma_start(out=xdo_t[:], in_=xdo2[g2])
        for sp in range(4):                    # pairs of c-subtiles
            ps_u = psU.tile([128, 2, KI], F32, tag="psu")
            for h in range(2):
                s = 2 * sp + h
                t = 8 * g2 + s
                nc.tensor.matmul(ps_u[:, h, :], lhsT=xdo_t[:, s, :],
                                 rhs=wt_t[:, s, :], start=True, stop=True)
                nc.tensor.matmul(ps_s1[h][:], lhsT=xtall_sb[:, t, :],
                                 rhs=wt_t[:, s, :],
                                 start=(t < 2), stop=(t >= NT - 2))
            dst = u_all[:, g2, 2 * sp:2 * sp + 2, :]
            if (4 * g2 + sp) % 2 == 0:
                nc.scalar.copy(dst, ps_u[:])
            else:
                nc.vector.tensor_copy(dst, ps_u[:])

    # ---- v1 = squash(s1 / K); replicate to all 128 partitions ----
    s1a = sm1.tile([BL, KI], F32, tag="s1a")
    nc.scalar.copy(s1a[:], ps_s1[0][:])
    s1sb = sm1.tile([BL, KI], F32, tag="s1sb")
    nc.vector.tensor_add(s1sb[:], s1a[:], ps_s1[1][:])
    v1 = _squash(nc, sm1, eps_t, s1sb, 1.0 / K)
    v1b = sm1.tile([BL, KI], BF16, tag="v1b")
    nc.vector.tensor_copy(v1b[:], v1[:].rearrange("p i k -> p (i k)"))
    nc.sync.dma_start(out=v1d[:], in_=v1b[:])
    v1d_ap = v1d[:]
    rep_src = bass.AP(tensor=v1d_ap.tensor, offset=v1d_ap.offset,
                      ap=[[0, 16], [KI, 8], [1, KI]])
    nc.sync.dma_start(out=v1rep[:], in_=rep_src)

    # ---- phase 2: routing chain per super-tile, software-pipelined so
    # the DVE works on tile g+1's mul/tree while Pool/ACT chew tile g ----
    ps_s2 = [psS.tile([BL, KI], F32, name=f"ps_s2_{h}") for h in range(2)]
    e_tiles = {}

    def stage_a(g2):
        u_g = u_all[:, g2]                     # [128, 8, KI]
        tmp = work.tile([128, 8, KI], BF16, tag="tmp")
        nc.vector.tensor_mul(tmp[:], u_g, _bcast_ins(v1rep[:], 8))
        t4 = tmp[:].rearrange("p s (i k) -> p s i k", k=K)
        f1 = workB.tile([128, 8, 8, K], BF16, tag="f1")
        nc.vector.tensor_add(f1[:], t4[:, :, 0:8, :], t4[:, :, 8:16, :])
        f2 = sm.tile([128, 8, 4, K], BF16, tag="f2")
        nc.gpsimd.tensor_add(f2[:], f1[:, :, 0:4, :], f1[:, :, 4:8, :])
        f3 = sm.tile([128, 8, 2, K], BF16, tag="f3")
        nc.gpsimd.tensor_add(f3[:], f2[:, :, 0:2, :], f2[:, :, 2:4, :])
        a_t = sm.tile([128, 8, K], F32, tag="a")
        nc.gpsimd.tensor_add(a_t[:], f3[:, :, 0, :], f3[:, :, 1, :])
        e_t = sm.tile([128, 8, K], BF16, tag="e")
        nc.scalar.activation(e_t[:], a_t[:],
                             mybir.ActivationFunctionType.Exp, scale=1.0)
        e_tiles[g2] = e_t

    def stage_b(g2):
        u_g = u_all[:, g2]
        e_t = e_tiles.pop(g2)
        den = sm.tile([128, 8], F32, tag="den")
        nc.vector.reduce_sum(den[:], e_t[:], axis=mybir.AxisListType.X)
        rcp = sm.tile([128, 8], F32, tag="rcp")
        nc.vector.reciprocal(rcp[:], den[:])
        cc = sm.tile([128, 8, K], BF16, tag="cc")
        nc.vector.tensor_mul(cc[:], e_t[:], _bcast_last(rcp[:], K))
        tmp2 = workB.tile([128, 8, I, K], BF16, tag="tmp2")
        nc.vector.tensor_mul(
            tmp2[:], u_g.rearrange("p s (i k) -> p s i k", k=K),
            _bcast_ins(cc[:], I))
        for s in range(8):
            nc.tensor.matmul(
                ps_s2[s % 2][:], lhsT=obd_sb[:],
                rhs=tmp2[:, s, :, :].rearrange("p i k -> p (i k)"),
                start=(g2 == 0 and s < 2), stop=(g2 == G2 - 1 and s >= 6))

    for g2 in range(G2):
        stage_a(g2)
        if g2 > 0:
            stage_b(g2 - 1)
    stage_b(G2 - 1)

    # ---- v2 = squash(s2) -> out ----
    s2a = sm1.tile([BL, KI], F32, tag="s1a")
    nc.scalar.copy(s2a[:], ps_s2[0][:])
    s2sb = sm1.tile([BL, KI], F32, tag="s2sb")
    nc.vector.tensor_add(s2sb[:], s2a[:], ps_s2[1][:])
    v2 = _squash(nc, sm1, eps_t, s2sb, 1.0)
    nc.sync.dma_start(out=out_d, in_=v2[:].rearrange("p i k -> p (i k)"))


_PROG = None


def _get_program():
    global _PROG
    if _PROG is None:
        nc = bacc.Bacc("TRN2", target_bir_lowering=False, debug=False,
                       num_devices=NC)
        wt_d = nc.dram_tensor("wt", [C * J, KI], BF16, kind="ExternalInput")
        xtall_d = nc.dram_tensor("xtall", [128, NT, BL], BF16,
                                 kind="ExternalInput")
        xdo2_d = nc.dram_tensor("xdo2", [G2, 128, 8, 128], BF16,
                                kind="ExternalInput")
        obd8_d = nc.dram_tensor("obd8", [128, BL], BF16, kind="ExternalInput")
        out_d = nc.dram_tensor("out", [BL, KI], F32, kind="ExternalOutput")
        with tile.TileContext(nc) as tc:
            _body(tc, wt_d[:], xtall_d[:], xdo2_d[:], obd8_d[:], out_d[:])
        nc.compile()
        _PROG = nc
    return _PROG


def _prep_inputs(x, W):
    import ml_dtypes
    bf = ml_dtypes.bfloat16
    # W [K, C, I, J] -> wt [(c,j), (i,k)]   (shared by all cores)
    wt = np.ascontiguousarray(W.transpose(1, 3, 2, 0)).reshape(C * J, KI)
    wt = wt.astype(bf)
    obd8 = np.zeros((16, BL, BL), np.float32)
    for b in range(BL):
        obd8[:, b, b] = 1.0
    obd8 = obd8.reshape(128, BL).astype(bf)
    in_maps = []
    for m in range(NC):
        xs = x[m * BL:(m + 1) * BL]                    # [BL, C, J]
        # xtall [(c'16,j8), t, b]
        xt5 = xs.reshape(BL, NT, 16, J)                # [b, t, c', j]
        xtall = np.ascontiguousarray(
            xt5.transpose(2, 3, 1, 0)).reshape(128, NT, BL).astype(bf)
        # xdo2 [g2, (c'16,j8), s, (c16,b8)]
        x6 = xs.reshape(BL, G2, 8, 16, J)              # [b, g2, s, c', j]
        xdo2 = np.zeros((G2, 16, J, 8, 16, BL), np.float32)
        for cp in range(16):
            # [b, g2, s, j] -> [g2, j, s, b]
            xdo2[:, cp, :, :, cp, :] = x6[:, :, :, cp, :].transpose(1, 3, 2, 0)
        xdo2 = xdo2.reshape(G2, 128, 8, 128).astype(bf)
        in_maps.append({"wt": wt, "xtall": xtall, "xdo2": xdo2, "obd8": obd8})
    return in_maps


def kernel(x, W):
    global LAST_RESULTS
    x = np.ascontiguousarray(np.asarray(x, np.float32))
    W = np.ascontiguousarray(np.asarray(W, np.float32))
    assert x.shape == (B, C, J) and W.shape == (K, C, I, J)
    nc = _get_program()
    in_maps = _prep_inputs(x, W)
    res = run_bass_kernel_spmd(nc, in_maps, core_ids=list(range(NC)),
                               trace=TRACE)
    LAST_RESULTS = res
    out = np.empty((B, K, I), np.float32)
    for m in range(NC):
        vm = np.asarray(res.results[m]["out"], np.float32).reshape(BL, I, K)
        out[m * BL:(m + 1) * BL] = vm.transpose(0, 2, 1)
    return np.ascontiguousarray(out)


# revision 36
# speedup vs baseline: 1.0827x; 1.0296x over previous
"""DenseCapsule routing (2 iterations) on 8 Trainium2 cores.

Sharding: data-parallel over batch (8 batch elems per core, W fully
replicated and STREAMED from DRAM).  Routing is independent per batch
element, so there are NO collectives - each core's span is just its own
work, which sidesteps the ~60us cross-core launch skew that any
AllReduce would have to wait out.

Math (ITERATIONS=2, v0=0 => logits after iter1 are 0, cc1 = 1/K):
  u[b,k,c,i]   = sum_j W[k,c,i,j] x[b,c,j]
  v1           = squash(sum_c u / K)
  a[b,k,c]     = sum_i u[b,k,c,i] v1[b,k,i]        (logits for iter 2)
  cc           = softmax_k(a)
  v2           = squash(sum_c cc[b,k,c] u[b,k,c,i])   -> output

Per-core layouts (host-prepped, BL = 8 local batch elems):
  wt    [(c,j)=16384, (i,k)=512]     u-matmul rhs + s1 rhs (streamed)
  xtall [(c'16,j8)=128, t=128, BL]   s1 lhsT per c-subtile t
  xdo2  [g2=16][(c'16,j8), s=8, (c16,b8)=128]
        block-diag x: = x[b, 128*g2+16*s+c, j] * (c==c')
        u-matmul lhsT -> psum_u[(c,b), (i,k)] = u[b,k,c,i]
  obd8  [(c16,b'8)=128, BL]          ones block-diag: delta(b'==b)
        s2 reduction lhsT: psum_s2[b,(i,k)] += sum_c tmp2[(c,b'),(i,k)]

Phase 1 streams W: per super-tile (128 c's) the PE does 8 u-matmuls +
8 s1-matmuls while ACT/DVE evict u to SBUF (u_all, 128KB/partition).
Phase 2 (after squash(s1) -> v1) runs the routing chain per super-tile
with the elementwise work split DVE/Pool and the c-reduction on PE.
"""

import numpy as np

import concourse.bacc as bacc
import concourse.bass as bass
import concourse.tile as tile
from concourse import mybir
from concourse._compat import with_exitstack
from concourse.bass_utils import run_bass_kernel_spmd

NC = 8
B = 64
BL = B // NC        # 8 local batch elements
C = 2048
J = 8
K = 32
I = 16
G2 = 16             # super-tiles of 128 c's
NT = C // 16        # 128 c-subtiles (16 c's -> 128 (c,j) rows)
KI = K * I          # 512
EPS = 1e-7

F32 = mybir.dt.float32
BF16 = mybir.dt.bfloat16

TRACE = False           # test.py sets True to capture NTFF timing
LAST_RESULTS = None     # BassKernelResults of the last run


def _bcast_last(ap, n):
    """Append a stride-0 dim of size n to an AP (free-dim broadcast)."""
    return bass.AP(tensor=ap.tensor, offset=ap.offset, ap=[*ap.ap, [0, n]])


def _bcast_ins(ap, n):
    """Insert a stride-0 dim of size n BEFORE the last free dim, keeping
    the innermost run packed (enables the DVE 2x perf mode)."""
    return bass.AP(tensor=ap.tensor, offset=ap.offset,
                   ap=[*ap.ap[:-1], [0, n], ap.ap[-1]])


def _squash(nc, pool, eps_t, s_sb, pre, out_dt=F32):
    """v = squash(pre * s_sb) for s_sb [BL, (i,k)] f32, squash over i."""
    s3 = s_sb[:].rearrange("p (i k) -> p i k", k=K)
    sq = pool.tile([BL, I, K], F32, tag="sq_sq")
    nc.vector.tensor_mul(sq[:], s3, s3)
    t1 = pool.tile([BL, 8, K], F32, tag="sq_t1")
    nc.vector.tensor_add(t1[:], sq[:, 0:8, :], sq[:, 8:16, :])
    t2 = pool.tile([BL, 4, K], F32, tag="sq_t2")
    nc.vector.tensor_add(t2[:], t1[:, 0:4, :], t1[:, 4:8, :])
    t3 = pool.tile([BL, 2, K], F32, tag="sq_t3")
    nc.vector.tensor_add(t3[:], t2[:, 0:2, :], t2[:, 2:4, :])
    n0 = pool.tile([BL, K], F32, tag="sq_n0")
    nc.vector.tensor_add(n0[:], t3[:, 0, :], t3[:, 1, :])
    sn = pool.tile([BL, K], F32, tag="sq_sn")
    nc.scalar.mul(sn[:], n0[:], pre * pre)          # |s|^2
    rt = pool.tile([BL, K], F32, tag="sq_rt")
    nc.scalar.activation(rt[:], sn[:], mybir.ActivationFunctionType.Sqrt,
                         bias=eps_t[:], scale=1.0)  # sqrt(|s|^2 + eps)
    dn = pool.tile([BL, K], F32, tag="sq_dn")
    nc.scalar.add(dn[:], sn[:], 1.0)                # 1 + |s|^2
    dd = pool.tile([BL, K], F32, tag="sq_dd")
    nc.vector.tensor_mul(dd[:], dn[:], rt[:])
    rc = pool.tile([BL, K], F32, tag="sq_rc")
    nc.vector.reciprocal(rc[:], dd[:])
    f0 = pool.tile([BL, K], F32, tag="sq_f0")
    nc.vector.tensor_mul(f0[:], sn[:], rc[:])
    g0 = pool.tile([BL, K], F32, tag="sq_g0")
    nc.scalar.mul(g0[:], f0[:], pre)                # scale applied to raw s_sb
    v = pool.tile([BL, I, K], out_dt, tag="sq_v")
    nc.vector.tensor_mul(v[:], s3, _bcast_ins(g0[:], I))
    return v


@with_exitstack
def _body(ctx, tc, wt, xtall, xdo2, obd8, out_d):
    nc = tc.nc
    singles = ctx.enter_context(tc.tile_pool(name="singles", bufs=1))
    wtp = ctx.enter_context(tc.tile_pool(name="wtp", bufs=2))
    xdop = ctx.enter_context(tc.tile_pool(name="xdop", bufs=2))
    psS = ctx.enter_context(tc.tile_pool(name="psS", bufs=1, space="PSUM"))
    psU = ctx.enter_context(tc.tile_pool(name="psU", bufs=2, space="PSUM"))
    work = ctx.enter_context(tc.tile_pool(name="work", bufs=1))
    workB = ctx.enter_context(tc.tile_pool(name="workB", bufs=2))
    sm = ctx.enter_context(tc.tile_pool(name="sm", bufs=2))
    sm1 = ctx.enter_context(tc.tile_pool(name="sm1", bufs=1))
    dram = ctx.enter_context(tc.tile_pool(name="dram", bufs=1, space="DRAM"))

    xtall_sb = singles.tile([128, NT, BL], BF16)
    nc.sync.dma_start(out=xtall_sb[:], in_=xtall)
    obd_sb = singles.tile([128, BL], BF16)
    nc.sync.dma_start(out=obd_sb[:], in_=obd8)
    eps_t = singles.tile([BL, 1], F32)
    nc.vector.memset(eps_t[:], EPS)
    u_all = singles.tile([128, G2, 8, KI], BF16)
    v1rep = singles.tile([128, KI], BF16)
    v1d = dram.tile([BL, KI], BF16, name="v1d")

    # ---- phase 1: stream W; u-matmuls + s1-matmuls; evict u ----
    # s1-matmuls are M=8, so 4 of them (4 c-subtiles) are packed into one
    # PE pass via col-tiling (tile_position=(0,32j), psum sliced at
    # base_partition 32j) - they run concurrently on 4 col-groups.
    ps_s1 = psS.tile([128, KI], F32, name="ps_s1")
    wt_ap = wt  # [C*J, KI] dram AP
    for g2 in range(G2):
        wt_t = wtp.tile([128, 8, KI], BF16, tag="wt")
        src = bass.AP(tensor=wt_ap.tensor,
                      offset=wt_ap.offset + (1024 * g2) * KI,
                      ap=[[KI, 128], [128 * KI, 8], [1, KI]])
        nc.sync.dma_start(out=wt_t[:], in_=src)
        xdo_t = xdop.tile([128, 8, 128], BF16, tag="xdo")
        nc.gpsimd.dma_start(out=xdo_t[:], in_=xdo2[g2])
        for q in range(2):
            for sp in range(2):                # pairs of c-subtiles
                ps_u = psU.tile([128, 2, KI], F32, tag="psu")
                for h in range(2):
                    s = 4 * q + 2 * sp + h
                    nc.tensor.matmul(ps_u[:, h, :], lhsT=xdo_t[:, s, :],
                                     rhs=wt_t[:, s, :], start=True, stop=True)
                dst = u_all[:, g2, 4 * q + 2 * sp:4 * q + 2 * sp + 2, :]
                if (4 * g2 + 2 * q + sp) % 2 == 0:
                    nc.scalar.copy(dst, ps_u[:])
                else:
                    nc.vector.tensor_copy(dst, ps_u[:])
            for j in range(4):                 # col-tiled s1 pack
                s = 4 * q + j
                nc.tensor.matmul(ps_s1[32 * j:32 * j + BL, :],
                                 lhsT=xtall_sb[:, 8 * g2 + s, :],
                                 rhs=wt_t[:, s, :],
                                 start=(g2 == 0 and q == 0),
                                 stop=(g2 == G2 - 1 and q == 1),
                                 tile_position=(0, 32 * j))

    # ---- v1 = squash(s1 / K); replicate to all 128 partitions ----
    s1a = sm1.tile([BL, KI], F32, tag="s1a")
    nc.scalar.copy(s1a[:], ps_s1[0:BL, :])
    s1b = sm1.tile([BL, KI], F32, tag="s1b")
    nc.vector.tensor_add(s1b[:], s1a[:], ps_s1[32:32 + BL, :])
    s1c = sm1.tile([BL, KI], F32, tag="s1a")
    nc.vector.tensor_add(s1c[:], s1b[:], ps_s1[64:64 + BL, :])
    s1sb = sm1.tile([BL, KI], F32, tag="s1b")
    nc.vector.tensor_add(s1sb[:], s1c[:], ps_s1[96:96 + BL, :])
    v1 = _squash(nc, sm1, eps_t, s1sb, 1.0 / K)
    v1b = sm1.tile([BL, KI], BF16, tag="v1b")
    nc.vector.tensor_copy(v1b[:], v1[:].rearrange("p i k -> p (i k)"))
    nc.sync.dma_start(out=v1d[:], in_=v1b[:])
    v1d_ap = v1d[:]
    rep_src = bass.AP(tensor=v1d_ap.tensor, offset=v1d_ap.offset,
                      ap=[[0, 16], [KI, 8], [1, KI]])
    nc.sync.dma_start(out=v1rep[:], in_=rep_src)

    # ---- phase 2: routing chain per super-tile, software-pipelined so
    # the DVE works on tile g+1's mul/tree while Pool/ACT chew tile g ----
    ps_s2 = psS.tile([128, KI], F32, name="ps_s2")
    e_tiles = {}

    def stage_a(g2):
        u_g = u_all[:, g2]                     # [128, 8, KI]
        tmp = work.tile([128, 8, KI], BF16, tag="tmp")
        nc.vector.tensor_mul(tmp[:], u_g, _bcast_ins(v1rep[:], 8))
        t4 = tmp[:].rearrange("p s (i k) -> p s i k", k=K)
        f1 = workB.tile([128, 8, 8, K], BF16, tag="f1")
        nc.vector.tensor_add(f1[:], t4[:, :, 0:8, :], t4[:, :, 8:16, :])
        f2 = sm.tile([128, 8, 4, K], BF16, tag="f2")
        nc.gpsimd.tensor_add(f2[:], f1[:, :, 0:4, :], f1[:, :, 4:8, :])
        f3 = sm.tile([128, 8, 2, K], BF16, tag="f3")
        nc.gpsimd.tensor_add(f3[:], f2[:, :, 0:2, :], f2[:, :, 2:4, :])
        a_t = sm.tile([128, 8, K], F32, tag="a")
        nc.vector.tensor_add(a_t[:], f3[:, :, 0, :], f3[:, :, 1, :])
        e_t = sm.tile([128, 8, K], BF16, tag="e")
        nc.scalar.activation(e_t[:], a_t[:],
                             mybir.ActivationFunctionType.Exp, scale=1.0)
        e_tiles[g2] = e_t

    def stage_b(g2):
        u_g = u_all[:, g2]
        e_t = e_tiles.pop(g2)
        den = sm.tile([128, 8], F32, tag="den")
        nc.vector.reduce_sum(den[:], e_t[:], axis=mybir.AxisListType.X)
        rcp = sm.tile([128, 8], F32, tag="rcp")
        nc.vector.reciprocal(rcp[:], den[:])
        cc = sm.tile([128, 8, K], BF16, tag="cc")
        nc.vector.tensor_mul(cc[:], e_t[:], _bcast_last(rcp[:], K))
        tmp2 = workB.tile([128, 8, I, K], BF16, tag="tmp2")
        nc.vector.tensor_mul(
            tmp2[:], u_g.rearrange("p s (i k) -> p s i k", k=K),
            _bcast_ins(cc[:], I))
        for q in range(2):
            for j in range(4):                 # col-tiled s2 pack
                s = 4 * q + j
                nc.tensor.matmul(
                    ps_s2[32 * j:32 * j + BL, :], lhsT=obd_sb[:],
                    rhs=tmp2[:, s, :, :].rearrange("p i k -> p (i k)"),
                    start=(g2 == 0 and q == 0),
                    stop=(g2 == G2 - 1 and q == 1),
                    tile_position=(0, 32 * j))

    for g2 in range(G2):
        stage_a(g2)
        if g2 > 0:
            stage_b(g2 - 1)
    stage_b(G2 - 1)

    # ---- v2 = squash(s2) -> out ----
    s2a = sm1.tile([BL, KI], F32, tag="s1a")
    nc.scalar.copy(s2a[:], ps_s2[0:BL, :])
    s2b = sm1.tile([BL, KI], F32, tag="s1b")
    nc.vector.tensor_add(s2b[:], s2a[:], ps_s2[32:32 + BL, :])
    s2c = sm1.tile([BL, KI], F32, tag="s1a")
    nc.vector.tensor_add(s2c[:], s2b[:], ps_s2[64:64 + BL, :])
    s2sb = sm1.tile([BL, KI], F32, tag="s1b")
    nc.vector.tensor_add(s2sb[:], s2c[:], ps_s2[96:96 + BL, :])
    v2 = _squash(nc, sm1, eps_t, s2sb, 1.0)
    nc.sync.dma_start(out=out_d, in_=v2[:].rearrange("p i k -> p (i k)"))


_PROG = None


def _get_program():
    global _PROG
    if _PROG is None:
        nc = bacc.Bacc("TRN2", target_bir_lowering=False, debug=False,
                       num_devices=NC)
        wt_d = nc.dram_tensor("wt", [C * J, KI], BF16, kind="ExternalInput")
        xtall_d = nc.dram_tensor("xtall", [128, NT, BL], BF16,
                                 kind="ExternalInput")
        xdo2_d = nc.dram_tensor("xdo2", [G2, 128, 8, 128], BF16,
                                kind="ExternalInput")
        obd8_d = nc.dram_tensor("obd8", [128, BL], BF16, kind="ExternalInput")
        out_d = nc.dram_tensor("out", [BL, KI], F32, kind="ExternalOutput")
        with tile.TileContext(nc) as tc:
            _body(tc, wt_d[:], xtall_d[:], xdo2_d[:], obd8_d[:], out_d[:])
        nc.compile()
        _PROG = nc
    return _PROG


def _prep_inputs(x, W):
    import ml_dtypes
    bf = ml_dtypes.bfloat16
    # W [K, C, I, J] -> wt [(c,j), (i,k)]   (shared by all cores)
    wt = np.ascontiguousarray(W.transpose(1, 3, 2, 0)).reshape(C * J, KI)
    wt = wt.astype(bf)
    obd8 = np.zeros((16, BL, BL), np.float32)
    for b in range(BL):
        obd8[:, b, b] = 1.0
    obd8 = obd8.reshape(128, BL).astype(bf)
    in_maps = []
    for m in range(NC):
        xs = x[m * BL:(m + 1) * BL]                    # [BL, C, J]
        # xtall [(c'16,j8), t, b]
        xt5 = xs.reshape(BL, NT, 16, J)                # [b, t, c', j]
        xtall = np.ascontiguousarray(
            xt5.transpose(2, 3, 1, 0)).reshape(128, NT, BL).astype(bf)
        # xdo2 [g2, (c'16,j8), s, (c16,b8)]
        x6 = xs.reshape(BL, G2, 8, 16, J)              # [b, g2, s, c', j]
        xdo2 = np.zeros((G2, 16, J, 8, 16, BL), np.float32)
        for cp in range(16):
            # [b, g2, s, j] -> [g2, j, s, b]
            xdo2[:, cp, :, :, cp, :] = x6[:, :, :, cp, :].transpose(1, 3, 2, 0)
        xdo2 = xdo2.reshape(G2, 128, 8, 128).astype(bf)
        in_maps.append({"wt": wt, "xtall": xtall, "xdo2": xdo2, "obd8": obd8})
    return in_maps


def kernel(x, W):
    global LAST_RESULTS
    x = np.ascontiguousarray(np.asarray(x, np.float32))
    W = np.ascontiguousarray(np.asarray(W, np.float32))
    assert x.shape == (B, C, J) and W.shape == (K, C, I, J)
    nc = _get_program()
    in_maps = _prep_inputs(x, W)
    res = run_bass_kernel_spmd(nc, in_maps, core_ids=list(range(NC)),
                               trace=TRACE)
    LAST_RESULTS = res
    out = np.empty((B, K, I), np.float32)
    for m in range(NC):
        vm = np.asarray(res.results[m]["out"], np.float32).reshape(BL, I, K)
        out[m * BL:(m + 1) * BL] = vm.transpose(0, 2, 1)
    return np.ascontiguousarray(out)


# revision 38
# speedup vs baseline: 1.1462x; 1.0586x over previous
"""DenseCapsule routing (2 iterations) on 8 Trainium2 cores.

Sharding: data-parallel over batch (8 batch elems per core, W fully
replicated and STREAMED from DRAM).  Routing is independent per batch
element, so there are NO collectives - each core's span is just its own
work, which sidesteps the ~60us cross-core launch skew that any
AllReduce would have to wait out.

Math (ITERATIONS=2, v0=0 => logits after iter1 are 0, cc1 = 1/K):
  u[b,k,c,i]   = sum_j W[k,c,i,j] x[b,c,j]
  v1           = squash(sum_c u / K)
  a[b,k,c]     = sum_i u[b,k,c,i] v1[b,k,i]        (logits for iter 2)
  cc           = softmax_k(a)
  v2           = squash(sum_c cc[b,k,c] u[b,k,c,i])   -> output

Per-core layouts (host-prepped, BL = 8 local batch elems):
  wt    [(c,j)=16384, (i,k)=512]     u-matmul rhs + s1 rhs (streamed)
  xtall [(c'16,j8)=128, t=128, BL]   s1 lhsT per c-subtile t
  xdo2  [g2=16][(c'16,j8), s=8, (c16,b8)=128]
        block-diag x: = x[b, 128*g2+16*s+c, j] * (c==c')
        u-matmul lhsT -> psum_u[(c,b), (i,k)] = u[b,k,c,i]
  obd8  [(c16,b'8)=128, BL]          ones block-diag: delta(b'==b)
        s2 reduction lhsT: psum_s2[b,(i,k)] += sum_c tmp2[(c,b'),(i,k)]

Phase 1 streams W: per super-tile (128 c's) the PE does 8 u-matmuls +
8 s1-matmuls while ACT/DVE evict u to SBUF (u_all, 128KB/partition).
Phase 2 (after squash(s1) -> v1) runs the routing chain per super-tile
with the elementwise work split DVE/Pool and the c-reduction on PE.
"""

import numpy as np

import concourse.bacc as bacc
import concourse.bass as bass
import concourse.tile as tile
from concourse import mybir
from concourse._compat import with_exitstack
from concourse.bass_utils import run_bass_kernel_spmd

NC = 8
B = 64
BL = B // NC        # 8 local batch elements
C = 2048
J = 8
K = 32
I = 16
G2 = 16             # super-tiles of 128 c's
NT = C // 16        # 128 c-subtiles (16 c's -> 128 (c,j) rows)
KI = K * I          # 512
EPS = 1e-7

F32 = mybir.dt.float32
BF16 = mybir.dt.bfloat16

TRACE = False           # test.py sets True to capture NTFF timing
LAST_RESULTS = None     # BassKernelResults of the last run


def _bcast_last(ap, n):
    """Append a stride-0 dim of size n to an AP (free-dim broadcast)."""
    return bass.AP(tensor=ap.tensor, offset=ap.offset, ap=[*ap.ap, [0, n]])


def _bcast_ins(ap, n):
    """Insert a stride-0 dim of size n BEFORE the last free dim, keeping
    the innermost run packed (enables the DVE 2x perf mode)."""
    return bass.AP(tensor=ap.tensor, offset=ap.offset,
                   ap=[*ap.ap[:-1], [0, n], ap.ap[-1]])


def _squash(nc, pool, eps_t, s_sb, pre, out_dt=F32):
    """v = squash(pre * s_sb) for s_sb [BL, (i,k)] f32, squash over i."""
    s3 = s_sb[:].rearrange("p (i k) -> p i k", k=K)
    sq = pool.tile([BL, I, K], F32, tag="sq_sq")
    nc.vector.tensor_mul(sq[:], s3, s3)
    t1 = pool.tile([BL, 8, K], F32, tag="sq_t1")
    nc.vector.tensor_add(t1[:], sq[:, 0:8, :], sq[:, 8:16, :])
    t2 = pool.tile([BL, 4, K], F32, tag="sq_t2")
    nc.vector.tensor_add(t2[:], t1[:, 0:4, :], t1[:, 4:8, :])
    t3 = pool.tile([BL, 2, K], F32, tag="sq_t3")
    nc.vector.tensor_add(t3[:], t2[:, 0:2, :], t2[:, 2:4, :])
    n0 = pool.tile([BL, K], F32, tag="sq_n0")
    nc.vector.tensor_add(n0[:], t3[:, 0, :], t3[:, 1, :])
    sn = pool.tile([BL, K], F32, tag="sq_sn")
    nc.scalar.mul(sn[:], n0[:], pre * pre)          # |s|^2
    rt = pool.tile([BL, K], F32, tag="sq_rt")
    nc.scalar.activation(rt[:], sn[:], mybir.ActivationFunctionType.Sqrt,
                         bias=eps_t[:], scale=1.0)  # sqrt(|s|^2 + eps)
    dn = pool.tile([BL, K], F32, tag="sq_dn")
    nc.scalar.add(dn[:], sn[:], 1.0)                # 1 + |s|^2
    dd = pool.tile([BL, K], F32, tag="sq_dd")
    nc.vector.tensor_mul(dd[:], dn[:], rt[:])
    rc = pool.tile([BL, K], F32, tag="sq_rc")
    nc.vector.reciprocal(rc[:], dd[:])
    f0 = pool.tile([BL, K], F32, tag="sq_f0")
    nc.vector.tensor_mul(f0[:], sn[:], rc[:])
    g0 = pool.tile([BL, K], F32, tag="sq_g0")
    nc.scalar.mul(g0[:], f0[:], pre)                # scale applied to raw s_sb
    v = pool.tile([BL, I, K], out_dt, tag="sq_v")
    nc.vector.tensor_mul(v[:], s3, _bcast_ins(g0[:], I))
    return v


@with_exitstack
def _body(ctx, tc, wt, xtall, xdo2, obd8, out_d):
    nc = tc.nc
    singles = ctx.enter_context(tc.tile_pool(name="singles", bufs=1))
    wtp = ctx.enter_context(tc.tile_pool(name="wtp", bufs=2))
    xdop = ctx.enter_context(tc.tile_pool(name="xdop", bufs=2))
    psS = ctx.enter_context(tc.tile_pool(name="psS", bufs=1, space="PSUM"))
    psU = ctx.enter_context(tc.tile_pool(name="psU", bufs=2, space="PSUM"))
    work = ctx.enter_context(tc.tile_pool(name="work", bufs=1))
    workB = ctx.enter_context(tc.tile_pool(name="workB", bufs=2))
    sm = ctx.enter_context(tc.tile_pool(name="sm", bufs=2))
    sm1 = ctx.enter_context(tc.tile_pool(name="sm1", bufs=1))
    dram = ctx.enter_context(tc.tile_pool(name="dram", bufs=1, space="DRAM"))

    xtall_sb = singles.tile([128, NT, BL], BF16)
    nc.sync.dma_start(out=xtall_sb[:], in_=xtall)
    obd_sb = singles.tile([128, BL], BF16)
    nc.sync.dma_start(out=obd_sb[:], in_=obd8)
    eps_t = singles.tile([BL, 1], F32)
    nc.vector.memset(eps_t[:], EPS)
    u_all = singles.tile([128, G2, 8, KI], BF16)
    v1rep = singles.tile([128, KI], BF16)
    v1d = dram.tile([BL, KI], BF16, name="v1d")

    # ---- phase 1: stream W; u-matmuls + s1-matmuls; evict u ----
    # s1-matmuls are M=8, so 4 of them (4 c-subtiles) are packed into one
    # PE pass via col-tiling (tile_position=(0,32j), psum sliced at
    # base_partition 32j) - they run concurrently on 4 col-groups.
    ps_s1 = psS.tile([128, KI], F32, name="ps_s1")
    wt_ap = wt  # [C*J, KI] dram AP
    for g2 in range(G2):
        wt_t = wtp.tile([128, 8, KI], BF16, tag="wt")
        src = bass.AP(tensor=wt_ap.tensor,
                      offset=wt_ap.offset + (1024 * g2) * KI,
                      ap=[[KI, 128], [128 * KI, 8], [1, KI]])
        nc.sync.dma_start(out=wt_t[:], in_=src)
        xdo_t = xdop.tile([128, 8, 128], BF16, tag="xdo")
        nc.gpsimd.dma_start(out=xdo_t[:], in_=xdo2[g2])
        for q in range(2):
            for sp in range(2):                # pairs of c-subtiles
                ps_u = psU.tile([128, 2, KI], F32, tag="psu")
                for h in range(2):
                    s = 4 * q + 2 * sp + h
                    # xdo is block-diagonal: only the 4 diagonal 32x32
                    # blocks are nonzero -> 4 concurrent (K=32,M=32)
                    # tiled matmuls with tiny overlappable weight loads.
                    for i in range(4):
                        nc.tensor.matmul(
                            ps_u[32 * i:32 * i + 32, h, :],
                            lhsT=xdo_t[32 * i:32 * i + 32, s,
                                       32 * i:32 * i + 32],
                            rhs=wt_t[32 * i:32 * i + 32, s, :],
                            start=True, stop=True,
                            tile_position=(32 * i, 32 * i))
                dst = u_all[:, g2, 4 * q + 2 * sp:4 * q + 2 * sp + 2, :]
                if (4 * g2 + 2 * q + sp) % 2 == 0:
                    nc.scalar.copy(dst, ps_u[:])
                else:
                    nc.vector.tensor_copy(dst, ps_u[:])
            for j in range(4):                 # col-tiled s1 pack
                s = 4 * q + j
                nc.tensor.matmul(ps_s1[32 * j:32 * j + BL, :],
                                 lhsT=xtall_sb[:, 8 * g2 + s, :],
                                 rhs=wt_t[:, s, :],
                                 start=(g2 == 0 and q == 0),
                                 stop=(g2 == G2 - 1 and q == 1),
                                 tile_position=(0, 32 * j))

    # ---- v1 = squash(s1 / K); replicate to all 128 partitions ----
    s1a = sm1.tile([BL, KI], F32, tag="s1a")
    nc.scalar.copy(s1a[:], ps_s1[0:BL, :])
    s1b = sm1.tile([BL, KI], F32, tag="s1b")
    nc.vector.tensor_add(s1b[:], s1a[:], ps_s1[32:32 + BL, :])
    s1c = sm1.tile([BL, KI], F32, tag="s1a")
    nc.vector.tensor_add(s1c[:], s1b[:], ps_s1[64:64 + BL, :])
    s1sb = sm1.tile([BL, KI], F32, tag="s1b")
    nc.vector.tensor_add(s1sb[:], s1c[:], ps_s1[96:96 + BL, :])
    v1 = _squash(nc, sm1, eps_t, s1sb, 1.0 / K)
    v1b = sm1.tile([BL, KI], BF16, tag="v1b")
    nc.vector.tensor_copy(v1b[:], v1[:].rearrange("p i k -> p (i k)"))
    nc.sync.dma_start(out=v1d[:], in_=v1b[:])
    v1d_ap = v1d[:]
    rep_src = bass.AP(tensor=v1d_ap.tensor, offset=v1d_ap.offset,
                      ap=[[0, 16], [KI, 8], [1, KI]])
    nc.sync.dma_start(out=v1rep[:], in_=rep_src)

    # ---- phase 2: routing chain per super-tile, software-pipelined so
    # the DVE works on tile g+1's mul/tree while Pool/ACT chew tile g ----
    ps_s2 = psS.tile([128, KI], F32, name="ps_s2")
    e_tiles = {}
    f3_tiles = {}

    def stage_a(g2):
        u_g = u_all[:, g2]                     # [128, 8, KI]
        tmp = work.tile([128, 8, KI], BF16, tag="tmp")
        nc.vector.tensor_mul(tmp[:], u_g, _bcast_ins(v1rep[:], 8))
        t4 = tmp[:].rearrange("p s (i k) -> p s i k", k=K)
        f1 = workB.tile([128, 8, 8, K], BF16, tag="f1")
        nc.vector.tensor_add(f1[:], t4[:, :, 0:8, :], t4[:, :, 8:16, :])
        f2 = sm.tile([128, 8, 4, K], BF16, tag="f2")
        nc.gpsimd.tensor_add(f2[:], f1[:, :, 0:4, :], f1[:, :, 4:8, :])
        f3 = sm.tile([128, 8, 2, K], BF16, tag="f3")
        nc.gpsimd.tensor_add(f3[:], f2[:, :, 0:2, :], f2[:, :, 2:4, :])
        f3_tiles[g2] = f3

    def stage_b1(g2):
        f3 = f3_tiles.pop(g2)
        a_t = sm.tile([128, 8, K], F32, tag="a")
        nc.vector.tensor_add(a_t[:], f3[:, :, 0, :], f3[:, :, 1, :])
        e_t = sm.tile([128, 8, K], BF16, tag="e")
        nc.scalar.activation(e_t[:], a_t[:],
                             mybir.ActivationFunctionType.Exp, scale=1.0)
        e_tiles[g2] = e_t

    def stage_b2(g2):
        u_g = u_all[:, g2]
        e_t = e_tiles.pop(g2)
        den = sm.tile([128, 8], F32, tag="den")
        nc.vector.reduce_sum(den[:], e_t[:], axis=mybir.AxisListType.X)
        rcp = sm.tile([128, 8], F32, tag="rcp")
        nc.vector.reciprocal(rcp[:], den[:])
        cc = sm.tile([128, 8, K], BF16, tag="cc")
        nc.vector.tensor_mul(cc[:], e_t[:], _bcast_last(rcp[:], K))
        tmp2 = workB.tile([128, 8, I, K], BF16, tag="tmp2")
        nc.vector.tensor_mul(
            tmp2[:], u_g.rearrange("p s (i k) -> p s i k", k=K),
            _bcast_ins(cc[:], I))
        for q in range(2):
            for j in range(4):                 # col-tiled s2 pack
                s = 4 * q + j
                nc.tensor.matmul(
                    ps_s2[32 * j:32 * j + BL, :], lhsT=obd_sb[:],
                    rhs=tmp2[:, s, :, :].rearrange("p i k -> p (i k)"),
                    start=(g2 == 0 and q == 0),
                    stop=(g2 == G2 - 1 and q == 1),
                    tile_position=(0, 32 * j))

    for g2 in range(G2):
        stage_a(g2)
        if g2 >= 1:
            stage_b1(g2 - 1)
        if g2 >= 2:
            stage_b2(g2 - 2)
    stage_b1(G2 - 1)
    stage_b2(G2 - 2)
    stage_b2(G2 - 1)

    # ---- v2 = squash(s2) -> out ----
    s2a = sm1.tile([BL, KI], F32, tag="s1a")
    nc.scalar.copy(s2a[:], ps_s2[0:BL, :])
    s2b = sm1.tile([BL, KI], F32, tag="s1b")
    nc.vector.tensor_add(s2b[:], s2a[:], ps_s2[32:32 + BL, :])
    s2c = sm1.tile([BL, KI], F32, tag="s1a")
    nc.vector.tensor_add(s2c[:], s2b[:], ps_s2[64:64 + BL, :])
    s2sb = sm1.tile([BL, KI], F32, tag="s1b")
    nc.vector.tensor_add(s2sb[:], s2c[:], ps_s2[96:96 + BL, :])
    v2 = _squash(nc, sm1, eps_t, s2sb, 1.0)
    nc.sync.dma_start(out=out_d, in_=v2[:].rearrange("p i k -> p (i k)"))


_PROG = None


def _get_program():
    global _PROG
    if _PROG is None:
        nc = bacc.Bacc("TRN2", target_bir_lowering=False, debug=False,
                       num_devices=NC)
        wt_d = nc.dram_tensor("wt", [C * J, KI], BF16, kind="ExternalInput")
        xtall_d = nc.dram_tensor("xtall", [128, NT, BL], BF16,
                                 kind="ExternalInput")
        xdo2_d = nc.dram_tensor("xdo2", [G2, 128, 8, 128], BF16,
                                kind="ExternalInput")
        obd8_d = nc.dram_tensor("obd8", [128, BL], BF16, kind="ExternalInput")
        out_d = nc.dram_tensor("out", [BL, KI], F32, kind="ExternalOutput")
        with tile.TileContext(nc) as tc:
            _body(tc, wt_d[:], xtall_d[:], xdo2_d[:], obd8_d[:], out_d[:])
        nc.compile()
        _PROG = nc
    return _PROG


def _prep_inputs(x, W):
    import ml_dtypes
    bf = ml_dtypes.bfloat16
    # W [K, C, I, J] -> wt [(c,j), (i,k)]   (shared by all cores)
    wt = np.ascontiguousarray(W.transpose(1, 3, 2, 0)).reshape(C * J, KI)
    wt = wt.astype(bf)
    obd8 = np.zeros((16, BL, BL), np.float32)
    for b in range(BL):
        obd8[:, b, b] = 1.0
    obd8 = obd8.reshape(128, BL).astype(bf)
    in_maps = []
    for m in range(NC):
        xs = x[m * BL:(m + 1) * BL]                    # [BL, C, J]
        # xtall [(c'16,j8), t, b]
        xt5 = xs.reshape(BL, NT, 16, J)                # [b, t, c', j]
        xtall = np.ascontiguousarray(
            xt5.transpose(2, 3, 1, 0)).reshape(128, NT, BL).astype(bf)
        # xdo2 [g2, (c'16,j8), s, (c16,b8)]
        x6 = xs.reshape(BL, G2, 8, 16, J)              # [b, g2, s, c', j]
        xdo2 = np.zeros((G2, 16, J, 8, 16, BL), np.float32)
        for cp in range(16):
            # [b, g2, s, j] -> [g2, j, s, b]
            xdo2[:, cp, :, :, cp, :] = x6[:, :, :, cp, :].transpose(1, 3, 2, 0)
        xdo2 = xdo2.reshape(G2, 128, 8, 128).astype(bf)
        in_maps.append({"wt": wt, "xtall": xtall, "xdo2": xdo2, "obd8": obd8})
    return in_maps


def kernel(x, W):
    global LAST_RESULTS
    x = np.ascontiguousarray(np.asarray(x, np.float32))
    W = np.ascontiguousarray(np.asarray(W, np.float32))
    assert x.shape == (B, C, J) and W.shape == (K, C, I, J)
    nc = _get_program()
    in_maps = _prep_inputs(x, W)
    res = run_bass_kernel_spmd(nc, in_maps, core_ids=list(range(NC)),
                               trace=TRACE)
    LAST_RESULTS = res
    out = np.empty((B, K, I), np.float32)
    for m in range(NC):
        vm = np.asarray(res.results[m]["out"], np.float32).reshape(BL, I, K)
        out[m * BL:(m + 1) * BL] = vm.transpose(0, 2, 1)
    return np.ascontiguousarray(out)
